# revision 18
# baseline (speedup 1.0000x reference)
"""DKVMN write-head memory update kernel for Trainium2 (8 NeuronCores).

Computes, for each batch row b:
    erase = sigmoid(control @ erase_W.T + erase_b)          # [B, D]
    add   = tanh(control @ add_W.T + add_b)                 # [B, D]
    new_memory[b,m,d] = memory[b,m,d] * (1 - ww[b,m]*erase[b,d]) + ww[b,m]*add[b,d]

Sharding: pure data parallel over batch B (4096 -> 512 per core), Linear
weights replicated.

The kernel is DMA-bound (memory in + new_memory out dominate), so all bulk
I/O is fp16 (the 2e-2 rel-err budget leaves ~25x margin): halves HBM traffic
versus fp32, putting the per-core DMA floor at ~95.5 us (32.4 MiB at
360 GB/s).  The host pre-transposes memory to [B, D, M] so that the m axis
is contiguous.  In that layout the update factors as

    u   = erase[b,d]*mem[b,d,m] - add[b,d]      (per-d two-scalar tensor_scalar)
    t   = u * (-ww[b,m])                        (tensor_tensor, ww broadcast over d)
    out = mem + t                               (PE identity-matmul PSUM accumulate)

Engine mapping (every engine stays well under the DMA roofline):
  - The per-d tensor_scalar hits the DVE 4x_2p fast path (the per-partition
    fp32 scalar operands are exempt from the 2-byte-dtype rule); a slice of
    each chunk's per-d ops runs on GPSIMD to keep DVE fed.
  - The ww pass hits DVE 2x_1p (the broadcast middle dim keeps the last dim
    packed).
  - The final add runs on the otherwise idle PE as an exact-fp32 PSUM
    accumulation: psum = I @ mem is issued right at load time (start=True),
    psum += I @ t in the tail (stop=True); ACT drains psum to fp16 SBUF.
  - -add comes free by negating the add-path weights/bias on the host (tanh
    is odd); -ww is negated on the host too, which also absorbs the sign of
    u = (mem*e) - a.

Pipelining: the W2/PE/drain/store tail is deferred by one chunk so each
engine's share of chunk c's per-d work overlaps the previous chunk's tail.
Loads are issued on SP (with the btile-0 ctrl load first, since the whole
compute pipeline gates on the first signals), stores on ACT right after the
psum drain.  For the last DRAIN_SS chunks the two psum halves are stored
separately and issued from SP: at that point no loads remain, the ACT
sequencer stays out of the store path, and the store drain runs at psum-half
granularity with no DMA gaps.  Buffer counts (BUFS_MEM/BUFS_OST) are sized
so neither the load prefetch nor the copy->store handoff ever throttles the
single DMA-engine pool; TimelineSim: ~100.2 us/core vs the ~95.5 us DMA
floor (startup ramp + final-store semaphore account for the difference).
"""

import sys

for _p in ("/opt/trn_rl_repo",):
    if _p not in sys.path:
        sys.path.insert(0, _p)

from contextlib import ExitStack

import numpy as np

import concourse.bass as bass
import concourse.tile as tile
from concourse import mybir

N_CORES = 8
B, M, D = 4096, 128, 128
B_LOC = B // N_CORES  # 512
P = 128               # SBUF partitions = batch tile
N_BTILES = B_LOC // P  # 4

F16 = mybir.dt.float16
F32 = mybir.dt.float32
ALU = mybir.AluOpType
ACTF = mybir.ActivationFunctionType

CD = 16        # d-slices per compute/DMA chunk -> [128, 16*128] fp16 = 512 KiB tiles
ND_ACT = 0     # of each chunk's CD per-d ops: this many on ACT,
ND_POOL = 6    # this many on GPSIMD, remainder on DVE
BUFS_MEM = 7
BUFS_WORK = 3
BUFS_OST = 5
PSUM_HALVES = 2
KPRE = 3       # mem loads issued before the one-time constants
COPY_ENG = "aa"   # psum-drain engine per half: a=ACT, p=Pool, v=DVE
SPLIT_STORE = 0   # 1: store each psum-half separately; 0: one store per chunk
CONST_ENG = "s"   # issue engine for per-btile ctrl/ww DMAs: a=ACT, s=SP
WT_ENG = "s"      # issue engine for one-time weight/bias/ident DMAs
STORE_ENG = "a"   # issue engine for output stores: a=ACT, s=SP, p=Pool
DRAIN_AV = 0      # last N chunks use drain_pat for the psum drain
DRAIN_PAT = "ap"  # copy-engine pattern for the drain chunks
START_CD = 0      # chunk size for the first btile (0 = same as CD)
DRAIN_SS = 10     # last N chunks store each psum-half separately
DRAIN_SE = 10     # last N chunks issue their stores from SP
DRAIN_QS = 0      # split each drain psum copy/store into this many pieces
DRAIN_CD = 0      # chunk size for the final btile (0 = same as CD)
SIG_FIRST = 0     # 1: btile-0 ctrl load before the first mem load


def legalize_waits(nc: bass.Bass) -> None:
    """Split multi-wait instructions for walrus.

    TRN2 codegen ('setupSyncWait: Too many sync wait commands') rejects
    instructions carrying more than one semaphore wait, but the Tile
    scheduler freely attaches several.  Hoist all but the last wait onto
    standalone single-wait sequencer instructions (InstEventSemaphore)
    inserted immediately before the instruction on the same engine."""
    for bb in nc.main_func.blocks:
        insts = bb.instructions
        if not any(
            i.sync_info is not None and i.sync_info.on_wait and len(i.sync_info.on_wait) > 1
            for i in insts
        ):
            continue
        new_list = []
        for inst in insts:
            si = inst.sync_info
            if si is not None and si.on_wait and len(si.on_wait) > 1:
                for w in si.on_wait[:-1]:
                    ev = mybir.InstEventSemaphore(
                        name=nc.get_next_instruction_name(),
                        engine=inst.engine,
                        ins=[],
                        outs=[],
                        sync_info=mybir.SyncInfo(on_wait=[w], on_update=[]),
                    )
                    nc.register_instruction(ev, overwrite=True)
                    new_list.append(ev)
                inst.sync_info = mybir.SyncInfo(
                    on_wait=[si.on_wait[-1]], on_update=list(si.on_update)
                )
            new_list.append(inst)
        bb.instructions = new_list


def build_nc(
    mode: str = "full",
    cd: int = CD,
    nd_act: int = ND_ACT,
    nd_pool: int = ND_POOL,
    bufs_mem: int = BUFS_MEM,
    bufs_work: int = BUFS_WORK,
    bufs_ost: int = BUFS_OST,
    halves: int = PSUM_HALVES,
    kpre: int = KPRE,
    copy_eng: str = COPY_ENG,
    split_store: int = SPLIT_STORE,
    const_eng: str = CONST_ENG,
    wt_eng: str = WT_ENG,
    store_eng: str = STORE_ENG,
    drain_av: int = DRAIN_AV,
    drain_cd: int = DRAIN_CD,
    drain_pat: str = DRAIN_PAT,
    start_cd: int = START_CD,
    drain_ss: int = DRAIN_SS,
    drain_se: int = DRAIN_SE,
    drain_qs: int = DRAIN_QS,
    sig_first: int = SIG_FIRST,
) -> bass.Bass:
    """mode: 'full' (real kernel), 'dma' (loads+stores only), 'load' (loads
    only), 'w1' (skip the tail).  All but 'full' produce WRONG output —
    timing bisection only."""
    assert D % cd == 0 and cd % halves == 0
    assert not drain_cd or (D % drain_cd == 0 and drain_cd % halves == 0 and (drain_cd // halves * M) % 512 == 0)
    assert not start_cd or (D % start_cd == 0 and start_cd % halves == 0 and (start_cd // halves * M) % 512 == 0)
    hd = cd // halves          # d-slices per psum tile
    assert (hd * M) % 512 == 0
    ncpb = D // cd             # chunks per btile

    nc = bass.Bass()

    # host-prepared inputs (fp16, memory transposed to [B, D, M]; the
    # add-path weights/bias and ww are negated on the host)
    memT_d = nc.dram_tensor("mem_t", [B_LOC, D, M], F16, kind="ExternalInput")
    wwn_d = nc.dram_tensor("ww_neg", [B_LOC, M], F16, kind="ExternalInput")
    ctrlT_d = nc.dram_tensor("ctrl_t", [D, B_LOC], F16, kind="ExternalInput")
    ewT_d = nc.dram_tensor("erase_w_t", [D, D], F16, kind="ExternalInput")
    awTn_d = nc.dram_tensor("add_w_t_neg", [D, D], F16, kind="ExternalInput")
    eb_d = nc.dram_tensor("erase_b", [D], F32, kind="ExternalInput")
    abn_d = nc.dram_tensor("add_b_neg", [D], F32, kind="ExternalInput")
    id_d = nc.dram_tensor("ident", [P, P], F16, kind="ExternalInput")
    out_d = nc.dram_tensor("new_memory", [B_LOC, D, M], F16, kind="ExternalOutput")

    with tile.TileContext(nc) as tc, ExitStack() as ctx:
        singles = ctx.enter_context(tc.tile_pool(name="singles", bufs=1))
        sig = ctx.enter_context(tc.tile_pool(name="sig", bufs=2))
        cload = ctx.enter_context(tc.tile_pool(name="cload", bufs=3))
        big = ctx.enter_context(tc.tile_pool(name="big", bufs=bufs_mem))
        work = ctx.enter_context(tc.tile_pool(name="work", bufs=bufs_work))
        ost = ctx.enter_context(tc.tile_pool(name="ost", bufs=bufs_ost))
        psum = ctx.enter_context(tc.tile_pool(name="psum", bufs=1, space="PSUM"))
        psacc = ctx.enter_context(tc.tile_pool(name="psacc", bufs=3, space="PSUM"))

        wt_tiles = {}
        bias_bc = {}
        ident_holder = [None]
        cdma = {"a": nc.scalar, "s": nc.sync, "p": nc.gpsimd}[const_eng]
        wdma = {"a": nc.scalar, "s": nc.sync, "p": nc.gpsimd}[wt_eng]
        sdma = {"a": nc.scalar, "s": nc.sync, "p": nc.gpsimd}[store_eng]

        def emit_consts():
            # ---- one-time constants (emitted after the first mem-load
            # prefetches so the big transfers hit the DMA engines first) ----
            # Linear weights + the PE identity, DVE-staged so each matmul's
            # waits funnel through one semaphore.
            for name, w_dram in (("e", ewT_d), ("a", awTn_d), ("i", id_d)):
                w_raw = cload.tile([D, D], F16, tag="wload")
                wdma.dma_start(out=w_raw[:], in_=w_dram[:, :])
                w_t = singles.tile([D, D], F16, tag=f"wt_{name}")
                nc.vector.tensor_copy(w_t[:], w_raw[:])
                wt_tiles[name] = w_t
            ident_holder[0] = wt_tiles["i"]

            # biases replicated across partitions via partition-broadcast DMA
            for name, b_dram in (("e", eb_d), ("a", abn_d)):
                b_t = singles.tile([P, D], F32, tag=f"bias_{name}")
                b_ap = bass.AP(tensor=b_dram[:].tensor, offset=0, ap=[[0, P], [1, D]])
                wdma.dma_start(out=b_t[:], in_=b_ap)
                bias_bc[name] = b_t

        def emit_sig_load(b0):
            """ctrl^T load + DVE staging for one batch tile."""
            ctrlT_raw = sig.tile([D, P], F16, tag="ctrl_raw")
            cdma.dma_start(out=ctrlT_raw[:], in_=ctrlT_d[:, b0 : b0 + P])
            ctrlT_sb = sig.tile([D, P], F16, tag="ctrl_stg")
            nc.vector.tensor_copy(ctrlT_sb[:], ctrlT_raw[:])
            return ctrlT_sb

        def emit_sigs(b0, ctrlT_sb=None):
            """erase / -add signals for one batch tile: psum = ctrl @ W.T,
            DVE adds the (partition-broadcast) bias, ACT applies the
            nonlinearity.  All later reads use [P,1] column APs (fp32)."""
            if ctrlT_sb is None:
                ctrlT_sb = emit_sig_load(b0)

            outs = {}
            for name, act_fn in (("e", ACTF.Sigmoid), ("a", ACTF.Tanh)):
                sig_ps = psum.tile([P, D], F32, tag=f"sig_{name}")
                nc.tensor.matmul(sig_ps[:], ctrlT_sb[:], wt_tiles[name][:])
                pre_sb = sig.tile([P, D], F32, tag=f"pre_{name}")
                nc.vector.tensor_tensor(
                    pre_sb[:], sig_ps[:], bias_bc[name][:], ALU.add
                )
                dst = sig.tile([P, D], F32, tag=f"sig_out_{name}")
                nc.scalar.activation(dst[:], pre_sb[:], act_fn)
                outs[name] = dst

            ww_sb = sig.tile([P, M], F16, tag="w")
            cdma.dma_start(out=ww_sb[:], in_=wwn_d[b0 : b0 + P, :])
            return outs["e"], outs["a"], ww_sb

        def emit_tail(mem_t, u_t, ww_sb, b0, d0, cdc, phs, ce_pat=None, ss=None, se=None, qs=0):
            """W2 (apply -ww), the PE psum accumulate of t (the mem pass ran
            at load time), the psum drain, and the store for one chunk."""
            ce_pat = ce_pat or copy_eng
            ss = split_store if ss is None else ss
            sd = {"a": nc.scalar, "s": nc.sync, "p": nc.gpsimd}[se] if se else sdma
            hdc = cdc // halves
            ww_bc = ww_sb[:].unsqueeze(1).broadcast_to((P, cdc, M))
            nc.vector.tensor_tensor(u_t, u_t, ww_bc, ALU.mult)
            ob = ost.tile([P, cdc, M], F16, tag="ost")
            for h in range(halves):
                ph = phs[h]
                for j in range(hdc * M // 512):
                    pj = ph[:, j * (512 // M) : (j + 1) * (512 // M), :]
                    s0 = h * hdc + j * (512 // M)
                    s1 = s0 + (512 // M)
                    nc.tensor.matmul(
                        pj, ident_holder[0][:], u_t[:, s0:s1, :],
                        start=False, stop=True,
                    )
                nq = qs if qs else 1
                qd = hdc // nq
                for q in range(nq):
                    ql = h * hdc + q * qd
                    oh = ob[:, ql : ql + qd, :]
                    ce = ce_pat[h % len(ce_pat)]
                    if ce == "a":
                        nc.scalar.activation(oh, ph[:, q * qd : (q + 1) * qd, :], ACTF.Copy)
                    elif ce == "p":
                        nc.gpsimd.tensor_copy(oh, ph[:, q * qd : (q + 1) * qd, :])
                    else:
                        nc.vector.tensor_copy(oh, ph[:, q * qd : (q + 1) * qd, :])
                    if ss:
                        sd.dma_start(
                            out=out_d[b0 : b0 + P, d0 + ql : d0 + ql + qd, :],
                            in_=oh,
                        )
            if not ss:
                sd.dma_start(
                    out=out_d[b0 : b0 + P, d0 : d0 + cdc, :], in_=ob[:]
                )

        def emit_mm1(mem_t, cdc):
            """PE pass-1: psum = I @ mem, issued right after the load so the
            tail only runs the accumulate pass.  Returns the psum tiles."""
            hdc = cdc // halves
            phs = []
            for h in range(halves):
                ph = psacc.tile([P, hdc, M], F32, tag="acc")
                for j in range(hdc * M // 512):
                    pj = ph[:, j * (512 // M) : (j + 1) * (512 // M), :]
                    s0 = h * hdc + j * (512 // M)
                    s1 = s0 + (512 // M)
                    nc.tensor.matmul(
                        pj, ident_holder[0][:], mem_t[:, s0:s1, :],
                        start=True, stop=False,
                    )
                phs.append(ph)
            return phs

        def emit_body():
            # chunk descriptors: (b0, d0, cd_c); the final btile is tapered
            # to drain_cd-sized chunks so the store drain runs at finer grain
            chunks = []
            for bt in range(N_BTILES):
                if bt == 0 and start_cd:
                    cdc = start_cd
                elif bt == N_BTILES - 1 and drain_cd:
                    cdc = drain_cd
                else:
                    cdc = cd
                for d0 in range(0, D, cdc):
                    chunks.append((bt * P, d0, cdc))
            pending = None
            e_sb = na_sb = ww_sb = None
            pre: dict = {}
            ctrl0 = None
            if mode in ("full", "w1") and sig_first:
                # btile 0's ctrl load + the signal-path constants go first:
                # the whole compute pipeline is gated on the first signals.
                ctrl0 = emit_sig_load(0)
            n_pre = min(kpre, len(chunks)) if mode in ("full", "w1") else 0
            for c in range(n_pre):
                b0, d0, cdc = chunks[c]
                mt = big.tile([P, cd, M], F16, tag="mem")
                nc.sync.dma_start(
                    out=mt[:, 0:cdc, :],
                    in_=memT_d[b0 : b0 + P, d0 : d0 + cdc, :],
                )
                pre[c] = mt
                if c == 0 and mode in ("full", "w1") and not sig_first:
                    ctrl0 = emit_sig_load(0)
            if mode in ("full", "w1"):
                emit_consts()
            for c, (b0, d0, cdc) in enumerate(chunks):
                if mode in ("full", "w1") and d0 == 0:
                    e_sb, na_sb, ww_sb = emit_sigs(b0, ctrl0 if b0 == 0 else None)

                if c in pre:
                    mem_t = pre.pop(c)
                else:
                    mem_t = big.tile([P, cd, M], F16, tag="mem")
                    nc.sync.dma_start(
                        out=mem_t[:, 0:cdc, :],
                        in_=memT_d[b0 : b0 + P, d0 : d0 + cdc, :],
                    )
                if mode == "load":
                    continue
                if mode == "dma":
                    nc.scalar.dma_start(
                        out=out_d[b0 : b0 + P, d0 : d0 + cdc, :],
                        in_=mem_t[:, 0:cdc, :],
                    )
                    continue

                phs = emit_mm1(mem_t, cdc) if mode == "full" else None

                # W1: u[:, d, :] = erase_d * mem[:, d, :] - add_d, split
                # across DVE / ACT / Pool (counts scaled to the chunk size).
                u_t = work.tile([P, cd, M], F16, tag="u")
                na_c = nd_act * cdc // cd
                np_c = nd_pool * cdc // cd
                for d in range(cdc):
                    dd = d0 + d
                    e_col = e_sb[:, dd : dd + 1]
                    na_col = na_sb[:, dd : dd + 1]
                    if d < na_c:
                        nc.scalar.activation(
                            u_t[:, d, :],
                            mem_t[:, d, :],
                            ACTF.Identity,
                            bias=na_col,
                            scale=e_col,
                        )
                    elif d < na_c + np_c:
                        nc.gpsimd.tensor_scalar(
                            out=u_t[:, d, :],
                            in0=mem_t[:, d, :],
                            scalar1=e_col,
                            scalar2=na_col,
                            op0=ALU.mult,
                            op1=ALU.add,
                        )
                    else:
                        nc.vector.tensor_scalar(
                            out=u_t[:, d, :],
                            in0=mem_t[:, d, :],
                            scalar1=e_col,
                            scalar2=na_col,
                            op0=ALU.mult,
                            op1=ALU.add,
                        )

                if mode == "w1":
                    if pending is not None:
                        nc.scalar.dma_start(
                            out=out_d[pending[3] : pending[3] + P,
                                      pending[4] : pending[4] + pending[5], :],
                            in_=pending[0][:, 0 : pending[5], :],
                        )
                    pending = (mem_t, u_t, ww_sb, b0, d0, cdc, phs)
                    continue

                if pending is not None:
                    n_left = len(chunks) - c
                    emit_tail(pending[0], pending[1][:, 0 : pending[5], :],
                              pending[2], pending[3], pending[4], pending[5],
                              pending[6],
                              ce_pat=drain_pat if n_left <= drain_av else None,
                              ss=1 if n_left <= drain_ss else None,
                              se="s" if n_left <= drain_se else None,
                              qs=drain_qs if n_left <= drain_ss else 0)
                pending = (mem_t, u_t, ww_sb, b0, d0, cdc, phs)

            if pending is not None:
                if mode == "w1":
                    nc.scalar.dma_start(
                        out=out_d[pending[3] : pending[3] + P,
                                  pending[4] : pending[4] + pending[5], :],
                        in_=pending[0][:, 0 : pending[5], :],
                    )
                else:
                    emit_tail(pending[0], pending[1][:, 0 : pending[5], :],
                              pending[2], pending[3], pending[4], pending[5],
                              pending[6],
                              ce_pat=drain_pat if drain_av else None,
                              ss=1 if drain_ss else None,
                              se="s" if drain_se else None,
                              qs=drain_qs if drain_ss else 0)

        emit_body()

    legalize_waits(nc)
    return nc


_CACHE: dict = {}


def _get_nc() -> bass.Bass:
    if "nc" not in _CACHE:
        _CACHE["nc"] = build_nc()
    return _CACHE["nc"]


def make_in_maps(**inputs) -> list:
    """Shard full inputs into per-core input maps (batch split, weights
    replicated).  memory is transposed to [B, D, M] and downcast to fp16 on
    the host; control_input and the Linear weights are pre-transposed; the
    add-path weights/bias and ww are negated (tanh is odd, so the device
    gets -add directly, and -ww makes the fused per-d op sign-correct)."""
    ci = np.asarray(inputs["control_input"], dtype=np.float32)
    mem = np.asarray(inputs["memory"], dtype=np.float32)
    ww = np.asarray(inputs["write_weight"], dtype=np.float32)
    ewT = np.ascontiguousarray(
        np.asarray(inputs["erase_W"], dtype=np.float32).T.astype(np.float16)
    )
    awTn = np.ascontiguousarray(
        (-np.asarray(inputs["add_W"], dtype=np.float32).T).astype(np.float16)
    )
    eb = np.ascontiguousarray(np.asarray(inputs["erase_b"], dtype=np.float32))
    abn = np.ascontiguousarray(-np.asarray(inputs["add_b"], dtype=np.float32))
    mem16T = np.ascontiguousarray(mem.astype(np.float16).transpose(0, 2, 1))
    wwn16 = (-ww).astype(np.float16)
    ci16 = ci.astype(np.float16)
    ident = np.eye(P, dtype=np.float16)
    in_maps = []
    for c in range(N_CORES):
        sl = slice(c * B_LOC, (c + 1) * B_LOC)
        in_maps.append(
            {
                "mem_t": mem16T[sl],
                "ww_neg": wwn16[sl],
                "ctrl_t": np.ascontiguousarray(ci16[sl].T),
                "erase_w_t": ewT,
                "add_w_t_neg": awTn,
                "erase_b": eb,
                "add_b_neg": abn,
                "ident": ident,
            }
        )
    return in_maps


def run_sharded(trace: bool = False, **inputs):
    """Run on all 8 cores; returns (full_output, BassKernelResults)."""
    from concourse.bass_utils import run_bass_kernel_spmd

    nc = _get_nc()
    res = run_bass_kernel_spmd(
        nc, make_in_maps(**inputs), core_ids=list(range(N_CORES)), trace=trace
    )
    out16 = np.concatenate(
        [res.results[c]["new_memory"] for c in range(N_CORES)], axis=0
    )
    out = np.ascontiguousarray(
        out16.astype(np.float32).transpose(0, 2, 1)
    )
    return out, res


def kernel(**inputs) -> np.ndarray:
    out, _ = run_sharded(trace=False, **inputs)
    return out

